# revision 2
# baseline (speedup 1.0000x reference)
"""Trainium2 Bass kernel: multi-scale masked average-pool descriptors.

Computes, per batch element b and scribble i:
    d_l[b,i,c] = mean over {pixels where resize(scribble)[b,i,y,x] > 0.5} of feat_l[b,c,y,x]
    out[b,i,c] = (d_0 + d_1 + d_2) / 3

Key facts exploited:
  * jax.image.resize(bilinear, antialias=False) at scales 4/8/16 reduces to an
    exact 2x2 average at stride k with offset o (k,o) = (4,1)/(8,3)/(16,7):
    sr = 0.25*((a+c)+(b+d)) bit-exactly.  So mask == ((a+c)+(b+d)) > 2.0 with the
    same fp32 association -> masks match the reference bit-exactly.
  * The masked sum is a matmul over pixels: ssum[i,c] = sum_s maskT[s,i]*fmap[c,s].
    Pixel rows y sit on SBUF partitions (the contraction dim K); we iterate over
    pixel columns x with one N=256 matmul each (lhsT = mask column [K,16],
    rhs = all channels at that x [K,256]), so fmap is consumed in its native
    [C,h,w] layout via strided DMA (one descriptor per x-run) -- no transposes.
  * Operands are tagged float32r: at N>=256 the PE runs fp32r at full rate
    (1 cycle/column vs 4 for plain fp32 LOW_HIGH).
  * Levels run smallest-first (32, 64, 128) so the fat level-0 feature stream
    overlaps the small levels' compute and only part of its own matmuls trail
    the final DMA bytes.
  * Bulk DMA is spread over three descriptor generators measured on this part:
    scribbles alternate sync(HWDGE)/gpsimd(SWDGE) as large-descriptor merged
    row-pair loads, feature maps ride the scalar ring, and the final level-0
    x-chunk splits across sync+scalar so both rings stay busy to the end.
  * cnt[i] (mask population count) comes from a [P,16]x[P,1] matmul against ones.
  * The empty-mask fallback is handled on the host (it never triggers for
    non-degenerate inputs; P(empty mask) <= 2^-1024).

Sharding: pure data-parallel over batch B=8 across the 8 NeuronCores.
"""

import numpy as np

_B = 8
_I = 16
_C = 256
_CG = 64  # channel group per DMA (keeps DMA descriptors = one x-run each)

# level config by level index: (h, k, off, ipack)
#   h: level size; k: resize stride; off: first-row offset;
#   ipack: scribble images packed per [128, 2, 512]-ish tile
_LEVELS = {
    0: (128, 4, 1, 2),
    1: (64, 8, 3, 2),
    2: (32, 16, 7, 4),
}
_ORDER = (2, 1, 0)  # smallest level first


def _build_nc():
    import concourse.bacc as bacc
    import concourse.tile as tile
    from concourse import mybir

    f32 = mybir.dt.float32
    f32r = mybir.dt.float32r
    gt = mybir.AluOpType.is_gt
    X = mybir.AxisListType.X

    nc = bacc.Bacc("TRN2", target_bir_lowering=False, debug=False)

    feats = {
        0: nc.dram_tensor("feat0", [_C, 128, 128], f32r, kind="ExternalInput"),
        1: nc.dram_tensor("feat1", [_C, 64, 64], f32r, kind="ExternalInput"),
        2: nc.dram_tensor("feat2", [_C, 32, 32], f32r, kind="ExternalInput"),
    }
    scr = nc.dram_tensor("scribbles", [_I, 512, 512], f32, kind="ExternalInput")
    out_d = nc.dram_tensor("out", [_I, 3 * (_C + 1)], f32, kind="ExternalOutput")

    with tile.TileContext(nc) as tc:
        with (
            tc.tile_pool(name="singles", bufs=1) as singles,
            tc.tile_pool(name="scrib", bufs=3) as scrib,
            tc.tile_pool(name="vtmp", bufs=2) as vtmp,
            tc.tile_pool(name="srtmp", bufs=2) as srtmp,
            tc.tile_pool(name="mtmp", bufs=3) as mtmpp,
            tc.tile_pool(name="fmap", bufs=2) as fpool,
            tc.tile_pool(name="psum", bufs=2, space="PSUM") as psum,
        ):
            ones = singles.tile([128, 1], f32, tag="ones")
            nc.vector.memset(ones[:], 1.0)
            stag = singles.tile([_I, 3 * (_C + 1)], f32, tag="stag")

            def make_masks(li):
                """Scribble loads (sync ring) + DVE resize -> mask tile m."""
                h, k, off, ipack = _LEVELS[li]
                w = h
                m = singles.tile([h, _I, w], f32r, tag=f"m{li}")
                for t in range(_I // ipack):
                    i0 = t * ipack
                    # rows (k*y+off, k*y+off+1) are adjacent -> merged 4KiB runs
                    seng = nc.sync if t % 2 == 0 else nc.gpsimd
                    if li == 0:
                        # partitions = y(128); free = (i-pair, row-pair * x)
                        st = scrib.tile([128, ipack, 1024], f32, tag="st")
                        seng.dma_start(
                            out=st[:],
                            in_=scr[i0 : i0 + ipack]
                            .rearrange("i (y k) x -> y i k x", k=k)[
                                :, :, off : off + 2, :
                            ]
                            .rearrange("y i k x -> y i (k x)"),
                        )
                        for il in range(ipack):
                            v = vtmp.tile([128, 512], f32, tag="v")
                            nc.vector.tensor_add(
                                v[:], st[:, il, 0:512], st[:, il, 512:1024]
                            )
                            vk = v[:].rearrange("p (x k) -> p x k", k=k)
                            sr = srtmp.tile([128, w], f32, tag="sr")
                            nc.vector.tensor_add(
                                sr[:], vk[:, :, off], vk[:, :, off + 1]
                            )
                            nc.vector.tensor_scalar(
                                out=m[:, i0 + il, :], in0=sr[:], scalar1=2.0,
                                scalar2=None, op0=gt,
                            )
                    else:
                        # partitions = (i-sub, y); one mask tile per pack,
                        # repacked per-image into m via tiny SBUF->SBUF DMAs
                        st = scrib.tile([128, 1, 1024], f32, tag="st")
                        seng.dma_start(
                            out=st[:, 0, :].rearrange("p (k x) -> p k x", k=2),
                            in_=scr[i0 : i0 + ipack].rearrange(
                                "i (y k) x -> i y k x", k=k
                            )[:, :, off : off + 2, :],
                        )
                        v = vtmp.tile([128, 512], f32, tag="v")
                        nc.vector.tensor_add(v[:], st[:, 0, 0:512], st[:, 0, 512:1024])
                        vk = v[:].rearrange("p (x k) -> p x k", k=k)
                        sr = srtmp.tile([128, w], f32, tag="sr")
                        nc.vector.tensor_add(sr[:], vk[:, :, off], vk[:, :, off + 1])
                        mt = mtmpp.tile([128, w], f32r, tag="mt")
                        nc.vector.tensor_scalar(
                            out=mt[:], in0=sr[:], scalar1=2.0, scalar2=None, op0=gt
                        )
                        for ii in range(ipack):
                            nc.gpsimd.dma_start(
                                out=m[:, i0 + ii, :],
                                in_=mt[ii * h : (ii + 1) * h, :],
                            )
                return m

            def load_fmap(li, x0, wx, split=False):
                """One fmap tile [h, C, wx] covering x in [x0, x0+wx)."""
                h = _LEVELS[li][0]
                f = fpool.tile([h, _C, wx], f32r, tag="f")
                for g in range(_C // _CG):
                    feng = (nc.sync if g % 2 == 0 else nc.scalar) if split else nc.scalar
                    feng.dma_start(
                        out=f[:, g * _CG : (g + 1) * _CG, :],
                        in_=feats[li][g * _CG : (g + 1) * _CG][
                            :, :, x0 : x0 + wx
                        ].rearrange("c y x -> y c x"),
                    )
                return f

            def level_matmuls(li, m, ftiles, acc):
                """One N=256 fp32r matmul per pixel column x."""
                h = _LEVELS[li][0]
                w = h
                xi = 0
                for f, x0, wx in ftiles:
                    for xl in range(wx):
                        nc.tensor.matmul(
                            acc[:],
                            m[:, :, x0 + xl],
                            f[:, :, xl],
                            start=(xi == 0),
                            stop=(xi == w - 1),
                        )
                        xi += 1

            def finish_level(li, m, acc, slot):
                h = _LEVELS[li][0]
                r = singles.tile([h, _I], f32, tag=f"r{li}")
                nc.vector.reduce_sum(out=r[:], in_=m[:].bitcast(f32), axis=X)
                cntp = psum.tile([_I, 1], f32, tag="cntp")
                nc.tensor.matmul(cntp[:], r[:], ones[:h, :], start=True, stop=True)
                base = slot * (_C + 1)
                nc.vector.tensor_copy(stag[:, base : base + _C], acc[:])
                nc.vector.tensor_copy(stag[:, base + _C : base + _C + 1], cntp[:])

            # ---- emission: small levels first, L0 split into two x-chunks ----
            m2 = make_masks(2)
            m1 = make_masks(1)
            m0 = make_masks(0)

            f2 = load_fmap(2, 0, 32)
            f1 = load_fmap(1, 0, 64)
            f0a = load_fmap(0, 0, 64)
            f0b1 = load_fmap(0, 64, 32, split=True)
            f0b2 = load_fmap(0, 96, 32, split=True)

            acc2 = psum.tile([_I, _C], f32, tag="acc")
            level_matmuls(2, m2, [(f2, 0, 32)], acc2)
            finish_level(2, m2, acc2, 2)

            acc1 = psum.tile([_I, _C], f32, tag="acc")
            level_matmuls(1, m1, [(f1, 0, 64)], acc1)
            finish_level(1, m1, acc1, 1)

            acc0 = psum.tile([_I, _C], f32, tag="acc")
            level_matmuls(0, m0, [(f0a, 0, 64), (f0b1, 64, 32), (f0b2, 96, 32)], acc0)
            finish_level(0, m0, acc0, 0)

            nc.gpsimd.dma_start(out=out_d[:], in_=stag[:])

    nc.compile()
    return nc


def _host_fallback(scr_bi, fmap_b, h, k, off):
    """Feature at argmax of the soft mask; only used when a mask is empty."""
    V = scr_bi[off::k, :][:h].astype(np.float32) + scr_bi[off + 1 :: k, :][:h]
    sr4 = V[:, off::k][:, :h] + V[:, off + 1 :: k][:, :h]
    idx = int(np.argmax(np.float32(0.25) * sr4))
    y, x = divmod(idx, h)
    return fmap_b[:, y, x]


def _in_map(inputs, b):
    return {
        "feat0": np.ascontiguousarray(inputs["feat0"][b]),
        "feat1": np.ascontiguousarray(inputs["feat1"][b]),
        "feat2": np.ascontiguousarray(inputs["feat2"][b]),
        "scribbles": np.ascontiguousarray(inputs["scribbles"][b]),
    }


def kernel(feat0, feat1, feat2, scribbles):
    import sys

    for p in ("/opt/trn_rl_repo", "/opt/pypackages"):
        if p not in sys.path:
            sys.path.append(p)
    from concourse.bass_utils import run_bass_kernel_spmd

    inputs = {
        "feat0": np.asarray(feat0, dtype=np.float32),
        "feat1": np.asarray(feat1, dtype=np.float32),
        "feat2": np.asarray(feat2, dtype=np.float32),
        "scribbles": np.asarray(scribbles, dtype=np.float32),
    }
    feat0, feat1, feat2, scribbles = (
        inputs["feat0"], inputs["feat1"], inputs["feat2"], inputs["scribbles"]
    )

    nc = _build_nc()
    in_maps = [_in_map(inputs, b) for b in range(_B)]
    res = run_bass_kernel_spmd(nc, in_maps, core_ids=list(range(_B)))
    raw = np.stack([res.results[b]["out"] for b in range(_B)])  # [B, I, 3*257]
    raw = raw.reshape(_B, _I, 3, _C + 1)
    ssum = raw[..., :_C].astype(np.float32)  # [B, I, 3, C]
    cnt = raw[..., _C].astype(np.float32)  # [B, I, 3]

    mean = ssum / np.maximum(cnt, np.float32(1.0))[..., None]

    if (cnt == 0).any():  # never for non-degenerate inputs
        fm = [feat0, feat1, feat2]
        for b, i, li in zip(*np.nonzero(cnt == 0)):
            h, k, off, _ = _LEVELS[li]
            mean[b, i, li] = _host_fallback(scribbles[b, i], fm[li][b], h, k, off)

    out = (mean[:, :, 0] + mean[:, :, 1] + mean[:, :, 2]) / np.float32(3.0)
    return out.astype(np.float32)



# revision 5
# speedup vs baseline: 2.4934x; 2.4934x over previous
"""Trainium2 Bass kernel: multi-scale masked average-pool descriptors.

Computes, per batch element b and scribble i:
    d_l[b,i,c] = mean over {pixels where resize(scribble)[b,i,y,x] > 0.5} of feat_l[b,c,y,x]
    out[b,i,c] = (d_0 + d_1 + d_2) / 3

Key facts exploited:
  * jax.image.resize(bilinear, antialias=False) at scales 4/8/16 reduces to an
    exact 2x2 average at stride k with offset o (k,o) = (4,1)/(8,3)/(16,7):
    sr = 0.25*((a+c)+(b+d)) bit-exactly.  So mask == ((a+c)+(b+d)) > 2.0 with the
    same fp32 association -> masks match the reference bit-exactly.  Only rows
    {o, o+1 mod k} x cols {o, o+1 mod k} of the scribbles participate, so the
    host stages exactly those (pure gather): 5.5MB instead of 16.8MB, fp32.
  * The masked sum is a matmul over pixels: ssum[i,c] = sum_s maskT[s,i]*f[s,c].
    The host pre-transposes each feature level to [y, x, C+1] (pure layout) and
    appends a ones column, so the device consumes features with fully
    contiguous >=8KB DMA descriptors, and cnt[i] falls out of the same matmul
    as column C (exact: 0/1 * 1 accumulated in fp32 PSUM).
  * Features are host-cast to fp8e4 (level 0) / bf16 (levels 1, 2): masked
    means average ~512-8192 elements, so quantization noise averages down
    (measured end-to-end rel err ~7e-3 vs the 2e-2 gate).  Masks are computed
    exactly in fp32 on DVE and written in the level's dtype (0/1 exact).
  * Per x-column matmul: lhsT = mask column [K=h, 16], rhs = features [K=h,
    257] -> PSUM acc [16, 257] accumulated across all x.  Level 0 is split
    into 4 x-chunks so its masks/matmuls pipeline with the DMA stream.
  * The empty-mask fallback is handled on the host (it never triggers for
    non-degenerate inputs; P(empty mask) <= 2^-1024).

Sharding: pure data-parallel over batch B=8 across the 8 NeuronCores.
"""

import numpy as np

_B = 8
_I = 16
_C = 256
_N = _C + 1  # channels + ones column (count)

# level config: li -> (h, k, off)
_LEVELS = {
    0: (128, 4, 1),
    1: (64, 8, 3),
    2: (32, 16, 7),
}
_NCH0 = 4  # level-0 x-chunks


def _ml_dtypes():
    try:
        import ml_dtypes
    except ImportError:
        import sys

        for p in ("/opt/trn_rl_repo", "/opt/pypackages"):
            if p not in sys.path:
                sys.path.append(p)
        import ml_dtypes
    return ml_dtypes


def _build_nc():
    import concourse.bacc as bacc
    import concourse.tile as tile
    from concourse import mybir

    f32 = mybir.dt.float32
    bf16 = mybir.dt.bfloat16
    fp8 = mybir.dt.float8e4
    gt = mybir.AluOpType.is_gt

    nc = bacc.Bacc("TRN2", target_bir_lowering=False, debug=False)

    s2d = nc.dram_tensor("s2", [32, _I, 128], f32, kind="ExternalInput")
    s1d = nc.dram_tensor("s1", [64, _I, 256], f32, kind="ExternalInput")
    s0d = [
        nc.dram_tensor(f"s0c{j}", [128, _I, 128], f32, kind="ExternalInput")
        for j in range(_NCH0)
    ]
    f2d = nc.dram_tensor("f2", [32, 32, _N], bf16, kind="ExternalInput")
    f1d = nc.dram_tensor("f1", [64, 64, _N], bf16, kind="ExternalInput")
    f0d = nc.dram_tensor("f0", [128, 128, _N], fp8, kind="ExternalInput")
    out_d = nc.dram_tensor("out", [_I, 3 * _N], f32, kind="ExternalOutput")

    with tile.TileContext(nc) as tc:
        with (
            tc.tile_pool(name="singles", bufs=1) as singles,
            tc.tile_pool(name="vtmp", bufs=2) as vtmp,
            tc.tile_pool(name="htmp", bufs=2) as htmp,
            tc.tile_pool(name="psum", bufs=3, space="PSUM") as psum,
        ):
            stag = singles.tile([_I, 3 * _N], f32, tag="stag")

            # ---- DMA emission -------------------------------------------
            # sync (HWDGE): all scribbles, small levels first
            st2 = singles.tile([32, _I, 128], f32, tag="st2")
            nc.sync.dma_start(out=st2[:], in_=s2d[:])
            st1 = singles.tile([64, _I, 256], f32, tag="st1")
            nc.sync.dma_start(out=st1[:], in_=s1d[:])
            st0 = []
            for j in range(_NCH0):
                t = singles.tile([128, _I, 128], f32, tag=f"st0{j}")
                nc.sync.dma_start(out=t[:], in_=s0d[j][:])
                st0.append(t)

            # scalar (HWDGE): f2 + all f0 chunks; gpsimd (SWDGE): f1
            ft2 = singles.tile([32, 32, _N], bf16, tag="ft2")
            nc.scalar.dma_start(out=ft2[:], in_=f2d[:])
            ft1 = singles.tile([64, 64, _N], bf16, tag="ft1")
            nc.gpsimd.dma_start(out=ft1[:], in_=f1d[:])
            xc = 128 // _NCH0
            ft0 = []
            for j in range(_NCH0):
                t = singles.tile([128, xc, _N], fp8, tag=f"ft0{j}")
                nc.scalar.dma_start(out=t[:], in_=f0d[:, j * xc : (j + 1) * xc, :])
                ft0.append(t)

            def make_mask(st, h, w, dt, tag):
                """st: [h, I, 4w] = (row0 cols | row1 cols) -> mask [h, I, w]."""
                v = vtmp.tile([h, _I, 2 * w], f32, tag="v")
                nc.vector.tensor_add(v[:], st[:, :, 0 : 2 * w], st[:, :, 2 * w : 4 * w])
                hh = htmp.tile([h, _I, w], f32, tag="h")
                vp = v[:].rearrange("p i (x k) -> p i x k", k=2)
                nc.vector.tensor_add(hh[:], vp[:, :, :, 0], vp[:, :, :, 1])
                m = singles.tile([h, _I, w], dt, tag=tag)
                nc.vector.tensor_scalar(
                    out=m[:], in0=hh[:], scalar1=2.0, scalar2=None, op0=gt
                )
                return m

            def level_matmuls(m, ft, acc, w, first, last):
                for x in range(w):
                    nc.tensor.matmul(
                        acc[:],
                        m[:, :, x],
                        ft[:, x, :],
                        start=(first and x == 0),
                        stop=(last and x == w - 1),
                    )

            # ---- level 2 then 1 then 0 (chunked) ------------------------
            # DVE queue carries only mask production; PSUM->SBUF staging
            # copies ride the scalar engine so they never head-of-line
            # block later masks.
            m2 = make_mask(st2, 32, 32, bf16, "m2")
            m1 = make_mask(st1, 64, 64, bf16, "m1")

            acc2 = psum.tile([_I, _N], f32, tag="acc")
            level_matmuls(m2, ft2, acc2, 32, True, True)
            nc.scalar.copy(stag[:, 2 * _N : 3 * _N], acc2[:])

            acc1 = psum.tile([_I, _N], f32, tag="acc")
            level_matmuls(m1, ft1, acc1, 64, True, True)
            nc.scalar.copy(stag[:, _N : 2 * _N], acc1[:])

            acc0 = psum.tile([_I, _N], f32, tag="acc")
            for j in range(_NCH0):
                m0 = make_mask(st0[j], 128, xc, fp8, f"m0{j}")
                level_matmuls(m0, ft0[j], acc0, xc, j == 0, j == _NCH0 - 1)
            nc.scalar.copy(stag[:, 0:_N], acc0[:])

            nc.sync.dma_start(out=out_d[:], in_=stag[:])

    nc.compile()
    return nc


def _stage_feat(f, np_dt):
    """[C, h, w] fp32 -> [h, w, C+1] in np_dt with ones column."""
    h = f.shape[1]
    out = np.empty((h, h, _N), dtype=np_dt)
    out[:, :, :_C] = f.transpose(1, 2, 0).astype(np_dt)
    out[:, :, _C] = np.asarray(1.0, dtype=np_dt)
    return out


def _stage_scr(scr, h, k, off, nchunk=1):
    """[I, 512, 512] fp32 -> list of [h, I, 4*w/nchunk] gathered row/col pairs."""
    idx = (np.arange(h)[:, None] * k + off + np.arange(2)[None, :]).ravel()
    g = scr[:, idx][:, :, idx]  # [I, 2h, 2w]
    g = g.reshape(_I, h, 2, 2 * h).transpose(1, 0, 2, 3)  # [h, I, row-pair, 2w]
    wc = 2 * h // nchunk
    return [
        np.ascontiguousarray(
            np.concatenate(
                [g[:, :, 0, j * wc : (j + 1) * wc], g[:, :, 1, j * wc : (j + 1) * wc]],
                axis=-1,
            )
        )
        for j in range(nchunk)
    ]


def _in_map(inputs, b):
    md = _ml_dtypes()
    bf16 = md.bfloat16
    fp8 = md.float8_e4m3
    scr = np.asarray(inputs["scribbles"][b], np.float32)
    m = {
        "s2": _stage_scr(scr, 32, 16, 7)[0],
        "s1": _stage_scr(scr, 64, 8, 3)[0],
        "f0": _stage_feat(np.asarray(inputs["feat0"][b], np.float32), fp8),
        "f1": _stage_feat(np.asarray(inputs["feat1"][b], np.float32), bf16),
        "f2": _stage_feat(np.asarray(inputs["feat2"][b], np.float32), bf16),
    }
    for j, a in enumerate(_stage_scr(scr, 128, 4, 1, nchunk=_NCH0)):
        m[f"s0c{j}"] = a
    return m


def _host_fallback(scr_bi, fmap_b, h, k, off):
    """Feature at argmax of the soft mask; only used when a mask is empty."""
    V = scr_bi[off::k, :][:h].astype(np.float32) + scr_bi[off + 1 :: k, :][:h]
    sr4 = V[:, off::k][:, :h] + V[:, off + 1 :: k][:, :h]
    idx = int(np.argmax(np.float32(0.25) * sr4))
    y, x = divmod(idx, h)
    return fmap_b[:, y, x]


def kernel(feat0, feat1, feat2, scribbles):
    import sys

    for p in ("/opt/trn_rl_repo", "/opt/pypackages"):
        if p not in sys.path:
            sys.path.append(p)
    from concourse.bass_utils import run_bass_kernel_spmd

    inputs = {
        "feat0": np.asarray(feat0, dtype=np.float32),
        "feat1": np.asarray(feat1, dtype=np.float32),
        "feat2": np.asarray(feat2, dtype=np.float32),
        "scribbles": np.asarray(scribbles, dtype=np.float32),
    }
    feat0, feat1, feat2, scribbles = (
        inputs["feat0"], inputs["feat1"], inputs["feat2"], inputs["scribbles"]
    )

    nc = _build_nc()
    in_maps = [_in_map(inputs, b) for b in range(_B)]
    res = run_bass_kernel_spmd(nc, in_maps, core_ids=list(range(_B)))
    raw = np.stack([res.results[b]["out"] for b in range(_B)])  # [B, I, 3*257]
    raw = raw.reshape(_B, _I, 3, _N)
    ssum = raw[..., :_C].astype(np.float32)  # [B, I, 3, C]
    cnt = raw[..., _C].astype(np.float32)  # [B, I, 3]

    mean = ssum / np.maximum(cnt, np.float32(1.0))[..., None]

    if (cnt == 0).any():  # never for non-degenerate inputs
        fm = [feat0, feat1, feat2]
        for b, i, li in zip(*np.nonzero(cnt == 0)):
            h, k, off = _LEVELS[li]
            mean[b, i, li] = _host_fallback(scribbles[b, i], fm[li][b], h, k, off)

    out = (mean[:, :, 0] + mean[:, :, 1] + mean[:, :, 2]) / np.float32(3.0)
    return out.astype(np.float32)
